# revision 13
# baseline (speedup 1.0000x reference)
"""LIF (leaky integrate-and-fire) spiking-neuron scan on 8 Trainium2 NeuronCores.

Reference semantics (per element, f32):
    h_t = v_{t-1} + (x_t - v_{t-1}) / 2        (tau = 2, v_reset = 0)
    s_t = (h_t >= 1)                           (spike, threshold v_th = 1)
    v_t = h_t * (1 - s_t)                      (hard reset)

Device formulation: shifted pre-activation u_t = v_{t-1} + x_t - 2, so
s_t = (u_t >= 0) and the whole step is ONE fused custom-DVE op:
    u_t = (0.5 * u_{t-1} + 1) * (u_{t-1} < 0) + (x_t - 2)
(x - 2 is precomputed on host; u_0 = -2 encodes v_0 = 0.)  The custom op
(registered into concourse.dve_ops at import, compiled into the per-NEFF
DVE table) runs at 1 elem/cycle/lane fp32 — one ~691 ns instruction per
timestep, ~727 ns dependent-chain pitch, bit-exact f32 arithmetic.

Output: u_t cast fp32 -> fp8e4 by the (otherwise idle) ACT engine, then
stored via HWDGE.  Spike <=> u_t >= 0 <=> fp8 sign bit clear (sign
survives rounding and saturation), so the host decodes
spikes = (u8_bits < 0x80).  Casting on ACT instead of during the DMA
keeps the 16.8 MB of fp32 u-reads off the SDMA/SBUF-AXI budget: DMA
moves only 16.8 MB in + 4.2 MB out per core vs 46.5 us of DVE time.
Loads get a dedicated HWDGE ring (SP); casts + stores ride the ACT ring.

Sharding: batch dim B=64 split across 8 cores (8 rows each); time stays
local.  DRAM layout is partition-major [128, T*512] so every DMA segment
is contiguous per partition.
"""

import os
import numpy as np

T, B, N = 64, 64, 8192
NCORES = 8
BL = B // NCORES          # batch rows per core
P = 128                   # SBUF partitions
F = (BL * N) // P         # free elems per partition per step  (512)

# timestep chunking: small first chunks prime the pipeline, then steady-state
LOAD_CHUNKS = [1, 1, 2, 4] + [4] * 14
assert sum(LOAD_CHUNKS) == T
UC = 4                    # u-history chunk (timesteps per SBUF u buffer)
CAST = 4                  # timesteps per ACT fp32->fp8 cast op
TAPER = 1                 # cast granularity for the last TAIL steps
TAIL = 4                  # final steps cast one-by-one to shrink the tail

_built = {}


def _register_lif_op():
    from concourse import dve_ops
    from concourse.dve_spec import (
        Spec, Src0, Src1, C0, Zero, One, lower, _has_src1,
    )
    from concourse.dve_uop import DveOpSpec

    for op in dve_ops.OPS:
        if op.name == "LIF_STEP_ANT":
            return op

    body = (Src0 * C0 + One) * (Src0 < Zero) + Src1

    def ref(in0, in1, s0, s1, imm2):
        f = np.float32
        mask = (in0 < 0).astype(f)
        return ((in0 * f(s0) + f(1.0)) * mask + in1).astype(f)

    spec = Spec(body=body, reference=ref)
    name = "LIF_STEP_ANT"
    row = dve_ops._CUSTOM_DVE_ROW_BASE + len(dve_ops.OPS)
    shas = {}
    for ver in ("v3", "v4"):
        tmp = DveOpSpec(
            name=name, opcode=row, uops=lower(spec, ver=ver),
            rd1_en=_has_src1(spec),
        )
        shas[ver] = tmp.sha(ver)
    op = dve_ops.DveOp(name, spec, subdim=False, uops_sha=shas)
    dve_ops.OPS.append(op)
    dve_ops._SUB_OPCODE_FOR_NAME[name] = row
    dve_ops.CUSTOM_DVE_SPECS[name] = spec
    return op


def _build():
    if "nc" in _built:
        return _built["nc"]

    from contextlib import ExitStack
    import concourse.mybir as mybir
    from concourse import bacc, tile

    # Slim the kernel-exit choreography: the stock exit is
    # drain -> all_engine_barrier -> clear sems -> all_engine_barrier; the
    # trailing barrier only orders the sem clears against later instructions,
    # of which there are none at kernel end (~3us saved).
    from concourse.vector_clock import ScopedClock

    def _slim_drain_and_barrier(self, tick_clock, wait_clock):
        drain_inst = self.nc.sync.drain()
        wait_clock.add_sem_waits(
            drain_inst.ins, ScopedClock({None: tick_clock.global_clock})
        )
        self.nc.all_engine_barrier()
        popped = self.nc._tile_sem_poison_stack.pop()
        assert popped is self._sem_poison
        self.nc.clear_and_free_semaphores(list(self.sems.allocated().values()))

    tile.TileContext._drain_and_barrier = _slim_drain_and_barrier

    lif_op = _register_lif_op()

    nc = bacc.Bacc("TRN2", target_bir_lowering=False, debug=False)
    # partition-major layouts: [P, T*F] so per-partition bytes are contiguous
    x_ext = nc.dram_tensor("x", [P, T * F], mybir.dt.float32, kind="ExternalInput")
    u8_ext = nc.dram_tensor("u8", [P, T * F], mybir.dt.float8e4, kind="ExternalOutput")

    Copy = mybir.ActivationFunctionType.Copy

    # scratch target for the store-gate dummy DMA
    gate_ext = nc.dram_tensor("gate", [1, 1], mybir.dt.float32, kind="Internal")

    with tile.TileContext(nc) as tc:
        with ExitStack() as ctx:
            # x chunks and fp8 output tiles are fully resident (unique tags,
            # no recycling): load issues are never gated on buffer reuse, so
            # both HWDGE rings stream the whole 16 MiB input uninterrupted.
            xp = ctx.enter_context(tc.tile_pool(name="xp", bufs=1))
            up = ctx.enter_context(tc.tile_pool(name="up", bufs=3))
            cp = ctx.enter_context(tc.tile_pool(name="cp", bufs=1))
            ip = ctx.enter_context(tc.tile_pool(name="ip", bufs=1))

            u0 = ip.tile([P, F], mybir.dt.float32)
            nc.vector.memset(u0[:], -2.0)

            # issue all loads up front, alternating the two HWDGE rings
            x_tiles = []
            t0 = 0
            for i, ch in enumerate(LOAD_CHUNKS):
                xt = xp.tile([P, ch * F], mybir.dt.float32, tag=f"x{i}")
                dma_eng = nc.sync if i % 2 == 0 else nc.scalar
                dma_eng.dma_start(out=xt[:], in_=x_ext[:, t0 * F:(t0 + ch) * F])
                x_tiles.append((t0, ch, xt))
                t0 += ch

            C8S = 8         # timesteps per fp8 output tile / store
            prev = u0[:]
            uc = None
            c8_tiles = []   # (t_first, tile) pending output stores
            c8 = None
            for (t0, ch, xt) in x_tiles:
                for k in range(ch):
                    t = t0 + k
                    # final steps cast one-by-one so the post-compute tail
                    # is a single small cast + store
                    cast = CAST if t < T - TAIL else TAPER
                    if t % UC == 0:
                        uc = up.tile([P, UC * F], mybir.dt.float32, tag="uchunk")
                    cur = uc[:, (t % UC) * F:(t % UC + 1) * F]
                    nc.vector._custom_dve(
                        lif_op, out=cur, in0=prev,
                        in1=xt[:, k * F:(k + 1) * F], s0=0.5,
                    )
                    prev = cur
                    if t % C8S == 0:
                        c8 = cp.tile([P, C8S * F], mybir.dt.float8e4,
                                     tag=f"c8_{t // C8S}")
                        c8_tiles.append((t, c8))
                    if (t + 1) % cast == 0:
                        # ACT cast fp32 -> fp8e4 (sign-exact; |u| << 448)
                        uq = t % UC
                        cq = t % C8S
                        nc.scalar.activation(
                            c8[:, (cq - cast + 1) * F:(cq + 1) * F],
                            uc[:, (uq - cast + 1) * F:(uq + 1) * F],
                            Copy, bias=0.0, scale=1.0,
                        )

            # store gate: a dummy DMA whose semaphore wait (on the final x
            # chunk's load) blocks the sync queue, deferring every output
            # store until ALL input loads have finished.  Keeps the fp8
            # stores from stealing HBM/SBUF-AXI bandwidth during the load
            # phase; they burst at full rate under the last compute steps.
            last_xt = x_tiles[-1][2]
            nc.sync.dma_start(out=gate_ext[:, :], in_=last_xt[:1, :1])
            for (ct0, ct) in c8_tiles:
                if ct0 + C8S == T:
                    # split the final store so the exit drain only waits on
                    # a small last transfer
                    h = C8S // 2
                    nc.sync.dma_start(
                        out=u8_ext[:, ct0 * F:(ct0 + h) * F], in_=ct[:, :h * F]
                    )
                    nc.sync.dma_start(
                        out=u8_ext[:, (ct0 + h) * F:(ct0 + C8S) * F],
                        in_=ct[:, h * F:],
                    )
                else:
                    nc.sync.dma_start(
                        out=u8_ext[:, ct0 * F:(ct0 + C8S) * F], in_=ct[:],
                    )

    nc.compile()
    _built["nc"] = nc
    return nc


def _install_ntff_hook() -> bool:
    """Provide antenv.axon_hooks (absent in this image) so that
    run_bass_kernel_spmd(trace=True) can capture NTFF profiles via the
    ctypes hook that trn_agent_boot already implements."""
    try:
        from antenv.axon_hooks import get_axon_ntff_profile_hook  # noqa: F401
        return True
    except ImportError:
        pass
    try:
        import sys
        import types
        import antenv
        from trn_agent_boot.trn_boot import _ntff_profile_via_ctypes

        hook = _ntff_profile_via_ctypes("/opt/axon/libaxon_pjrt.so")
        if hook is None:
            return False
        mod = types.ModuleType("antenv.axon_hooks")
        state = {"hook": hook}
        mod.get_axon_ntff_profile_hook = lambda: state["hook"]
        mod.set_axon_ntff_profile_hook = lambda h: state.__setitem__("hook", h)
        sys.modules["antenv.axon_hooks"] = mod
        antenv.axon_hooks = mod
        return True
    except Exception:
        return False


def kernel(x: np.ndarray) -> np.ndarray:
    import concourse.bass_utils as bass_utils

    nc = _build()

    x = np.asarray(x)
    assert x.shape == (T, B, N) and x.dtype == np.float32

    xs = x - np.float32(2.0)
    in_maps = []
    for c in range(NCORES):
        # [T, BL*N] -> [T, P, F] -> [P, T, F] -> [P, T*F]  (partition-major)
        shard = (
            xs[:, c * BL:(c + 1) * BL, :]
            .reshape(T, P, F)
            .transpose(1, 0, 2)
            .reshape(P, T * F)
        )
        in_maps.append({"x": np.ascontiguousarray(shard)})

    trace = bool(int(os.environ.get("LIF_TRACE", "0")))
    if trace:
        trace = _install_ntff_hook()
        # artifact upload has no bucket in this container; neuter it
        bass_utils.upload_artifacts = lambda tmpdir: tmpdir

    try:
        res = bass_utils.run_bass_kernel_spmd(
            nc, in_maps, list(range(NCORES)), trace=trace
        )
    except Exception:
        if not trace:
            raise
        res = bass_utils.run_bass_kernel_spmd(
            nc, in_maps, list(range(NCORES)), trace=False
        )
    _built["last_result"] = res

    out = np.empty((T, B, N), np.float32)
    for c in range(NCORES):
        u8 = np.asarray(res.results[c]["u8"])          # fp8e4 [P, T*F]
        bits = u8.view(np.uint8).reshape(P, T, F).transpose(1, 0, 2)
        spikes = (bits < 0x80).astype(np.float32).reshape(T, BL, N)
        out[:, c * BL:(c + 1) * BL, :] = spikes
    return out


# revision 15
# speedup vs baseline: 1.0556x; 1.0556x over previous
"""LIF (leaky integrate-and-fire) spiking-neuron scan on 8 Trainium2 NeuronCores.

Reference semantics (per element, f32):
    h_t = v_{t-1} + (x_t - v_{t-1}) / 2        (tau = 2, v_reset = 0)
    s_t = (h_t >= 1)                           (spike, threshold v_th = 1)
    v_t = h_t * (1 - s_t)                      (hard reset)

Device formulation: shifted pre-activation u_t = v_{t-1} + x_t - 2, so
s_t = (u_t >= 0) and the whole step is ONE fused custom-DVE op:
    u_t = (0.5 * u_{t-1} + 1) * (u_{t-1} < 0) + (x_t - 2)
(x - 2 is precomputed on host; u_0 = -2 encodes v_0 = 0.)  The custom op
(registered into concourse.dve_ops at import, compiled into the per-NEFF
DVE table) runs at 1 elem/cycle/lane fp32 — one ~691 ns instruction per
timestep, ~727 ns dependent-chain pitch, bit-exact f32 arithmetic.

Output: u_t cast fp32 -> fp8e4 by the (otherwise idle) ACT engine, then
stored via HWDGE.  Spike <=> u_t >= 0 <=> fp8 sign bit clear (sign
survives rounding and saturation), so the host decodes
spikes = (u8_bits < 0x80).  Casting on ACT instead of during the DMA
keeps the 16.8 MB of fp32 u-reads off the SDMA/SBUF-AXI budget: DMA
moves only 16.8 MB in + 4.2 MB out per core vs 46.5 us of DVE time.
Loads get a dedicated HWDGE ring (SP); casts + stores ride the ACT ring.

Sharding: batch dim B=64 split across 8 cores (8 rows each); time stays
local.  DRAM layout is partition-major [128, T*512] so every DMA segment
is contiguous per partition.
"""

import os
import numpy as np

T, B, N = 64, 64, 8192
NCORES = 8
BL = B // NCORES          # batch rows per core
P = 128                   # SBUF partitions
F = (BL * N) // P         # free elems per partition per step  (512)

# timestep chunking: small first chunks prime the pipeline, then steady-state
LOAD_CHUNKS = [1, 1, 2, 4] + [4] * 14
assert sum(LOAD_CHUNKS) == T
UC = 4                    # u-history chunk (timesteps per SBUF u buffer)
CAST = 4                  # timesteps per ACT fp32->fp8 cast op
TAPER = 1                 # cast granularity for the last TAIL steps
TAIL = 4                  # final steps cast one-by-one to shrink the tail

_built = {}


def _register_lif_op():
    from concourse import dve_ops
    from concourse.dve_spec import (
        Spec, Src0, Src1, C0, Zero, One, lower, _has_src1,
    )
    from concourse.dve_uop import DveOpSpec

    for op in dve_ops.OPS:
        if op.name == "LIF_STEP_ANT":
            return op

    body = (Src0 * C0 + One) * (Src0 < Zero) + Src1

    def ref(in0, in1, s0, s1, imm2):
        f = np.float32
        mask = (in0 < 0).astype(f)
        return ((in0 * f(s0) + f(1.0)) * mask + in1).astype(f)

    spec = Spec(body=body, reference=ref)
    name = "LIF_STEP_ANT"
    row = dve_ops._CUSTOM_DVE_ROW_BASE + len(dve_ops.OPS)
    shas = {}
    for ver in ("v3", "v4"):
        tmp = DveOpSpec(
            name=name, opcode=row, uops=lower(spec, ver=ver),
            rd1_en=_has_src1(spec),
        )
        shas[ver] = tmp.sha(ver)
    op = dve_ops.DveOp(name, spec, subdim=False, uops_sha=shas)
    dve_ops.OPS.append(op)
    dve_ops._SUB_OPCODE_FOR_NAME[name] = row
    dve_ops.CUSTOM_DVE_SPECS[name] = spec
    return op


def _build():
    if "nc" in _built:
        return _built["nc"]

    from contextlib import ExitStack
    import concourse.mybir as mybir
    from concourse import bacc, tile

    # Slim the kernel-exit choreography: the stock exit is
    # drain -> all_engine_barrier -> clear sems -> all_engine_barrier; the
    # trailing barrier only orders the sem clears against later instructions,
    # of which there are none at kernel end (~3us saved).
    from concourse.vector_clock import ScopedClock

    def _slim_drain_and_barrier(self, tick_clock, wait_clock):
        drain_inst = self.nc.sync.drain()
        wait_clock.add_sem_waits(
            drain_inst.ins, ScopedClock({None: tick_clock.global_clock})
        )
        self.nc.all_engine_barrier()
        popped = self.nc._tile_sem_poison_stack.pop()
        assert popped is self._sem_poison
        self.nc.clear_and_free_semaphores(list(self.sems.allocated().values()))

    tile.TileContext._drain_and_barrier = _slim_drain_and_barrier

    lif_op = _register_lif_op()

    nc = bacc.Bacc("TRN2", target_bir_lowering=False, debug=False)
    # partition-major layouts: [P, T*F] so per-partition bytes are contiguous
    x_ext = nc.dram_tensor("x", [P, T * F], mybir.dt.float32, kind="ExternalInput")
    u8_ext = nc.dram_tensor("u8", [P, T * F], mybir.dt.float8e4, kind="ExternalOutput")

    Copy = mybir.ActivationFunctionType.Copy

    # scratch target for the store-gate dummy DMA
    gate_ext = nc.dram_tensor("gate", [1, 1], mybir.dt.float32, kind="Internal")

    with tile.TileContext(nc) as tc:
        with ExitStack() as ctx:
            # x chunks and fp8 output tiles are fully resident (unique tags,
            # no recycling): load issues are never gated on buffer reuse, so
            # both HWDGE rings stream the whole 16 MiB input uninterrupted.
            xp = ctx.enter_context(tc.tile_pool(name="xp", bufs=1))
            up = ctx.enter_context(tc.tile_pool(name="up", bufs=5))
            cp = ctx.enter_context(tc.tile_pool(name="cp", bufs=1))
            ip = ctx.enter_context(tc.tile_pool(name="ip", bufs=1))

            u0 = ip.tile([P, F], mybir.dt.float32)
            nc.vector.memset(u0[:], -2.0)

            # Loads alternate the two HWDGE rings.  A ring holds only ~4
            # in-flight transfers, and a dma_start whose ring is full BLOCKS
            # its issuing engine's queue — so only the first few ACT-ring
            # (scalar-engine) loads are issued up front; the rest are paced
            # through the compute loop between casts.  Blocking the SP ring
            # is harmless (nothing else lives on the sync queue except the
            # gated stores at the end).
            x_tiles = []
            scalar_loads = []
            t0 = 0
            for i, ch in enumerate(LOAD_CHUNKS):
                xt = xp.tile([P, ch * F], mybir.dt.float32, tag=f"x{i}")
                src = x_ext[:, t0 * F:(t0 + ch) * F]
                if i % 2 == 0:
                    nc.sync.dma_start(out=xt[:], in_=src)
                elif len(scalar_loads) < 3 and i < 6:
                    nc.scalar.dma_start(out=xt[:], in_=src)
                    scalar_loads.append(None)
                else:
                    scalar_loads.append((xt, src))
                x_tiles.append((t0, ch, xt))
                t0 += ch
            pending_scalar = [p for p in scalar_loads if p is not None]

            C8S = 8         # timesteps per fp8 output tile / store
            prev = u0[:]
            uc = None
            c8_tiles = []   # (t_first, tile) pending output stores
            c8 = None
            for (t0, ch, xt) in x_tiles:
                for k in range(ch):
                    t = t0 + k
                    # final steps cast one-by-one so the post-compute tail
                    # is a single small cast + store
                    cast = CAST if t < T - TAIL else TAPER
                    if t % UC == 0:
                        uc = up.tile([P, UC * F], mybir.dt.float32, tag="uchunk")
                    cur = uc[:, (t % UC) * F:(t % UC + 1) * F]
                    nc.vector._custom_dve(
                        lif_op, out=cur, in0=prev,
                        in1=xt[:, k * F:(k + 1) * F], s0=0.5,
                    )
                    prev = cur
                    if t % C8S == 0:
                        c8 = cp.tile([P, C8S * F], mybir.dt.float8e4,
                                     tag=f"c8_{t // C8S}")
                        c8_tiles.append((t, c8))
                    if (t + 1) % cast == 0:
                        # ACT cast fp32 -> fp8e4 (sign-exact; |u| << 448)
                        uq = t % UC
                        cq = t % C8S
                        nc.scalar.activation(
                            c8[:, (cq - cast + 1) * F:(cq + 1) * F],
                            uc[:, (uq - cast + 1) * F:(uq + 1) * F],
                            Copy, bias=0.0, scale=1.0,
                        )
                        if pending_scalar:
                            xt_p, src_p = pending_scalar.pop(0)
                            nc.scalar.dma_start(out=xt_p[:], in_=src_p)

            # store gate: a dummy DMA whose semaphore wait (on the final x
            # chunk's load) blocks the sync queue, deferring every output
            # store until ALL input loads have finished.  Keeps the fp8
            # stores from stealing HBM/SBUF-AXI bandwidth during the load
            # phase; they burst at full rate under the last compute steps.
            last_xt = x_tiles[-1][2]
            nc.sync.dma_start(out=gate_ext[:, :], in_=last_xt[:1, :1])
            for (ct0, ct) in c8_tiles:
                if ct0 + C8S == T:
                    # split the final store so the exit drain only waits on
                    # a small last transfer
                    h = C8S // 2
                    nc.sync.dma_start(
                        out=u8_ext[:, ct0 * F:(ct0 + h) * F], in_=ct[:, :h * F]
                    )
                    nc.sync.dma_start(
                        out=u8_ext[:, (ct0 + h) * F:(ct0 + C8S) * F],
                        in_=ct[:, h * F:],
                    )
                else:
                    nc.sync.dma_start(
                        out=u8_ext[:, ct0 * F:(ct0 + C8S) * F], in_=ct[:],
                    )

    nc.compile()
    _built["nc"] = nc
    return nc


def _install_ntff_hook() -> bool:
    """Provide antenv.axon_hooks (absent in this image) so that
    run_bass_kernel_spmd(trace=True) can capture NTFF profiles via the
    ctypes hook that trn_agent_boot already implements."""
    try:
        from antenv.axon_hooks import get_axon_ntff_profile_hook  # noqa: F401
        return True
    except ImportError:
        pass
    try:
        import sys
        import types
        import antenv
        from trn_agent_boot.trn_boot import _ntff_profile_via_ctypes

        hook = _ntff_profile_via_ctypes("/opt/axon/libaxon_pjrt.so")
        if hook is None:
            return False
        mod = types.ModuleType("antenv.axon_hooks")
        state = {"hook": hook}
        mod.get_axon_ntff_profile_hook = lambda: state["hook"]
        mod.set_axon_ntff_profile_hook = lambda h: state.__setitem__("hook", h)
        sys.modules["antenv.axon_hooks"] = mod
        antenv.axon_hooks = mod
        return True
    except Exception:
        return False


def kernel(x: np.ndarray) -> np.ndarray:
    import concourse.bass_utils as bass_utils

    nc = _build()

    x = np.asarray(x)
    assert x.shape == (T, B, N) and x.dtype == np.float32

    xs = x - np.float32(2.0)
    in_maps = []
    for c in range(NCORES):
        # [T, BL*N] -> [T, P, F] -> [P, T, F] -> [P, T*F]  (partition-major)
        shard = (
            xs[:, c * BL:(c + 1) * BL, :]
            .reshape(T, P, F)
            .transpose(1, 0, 2)
            .reshape(P, T * F)
        )
        in_maps.append({"x": np.ascontiguousarray(shard)})

    trace = bool(int(os.environ.get("LIF_TRACE", "0")))
    if trace:
        trace = _install_ntff_hook()
        # artifact upload has no bucket in this container; neuter it
        bass_utils.upload_artifacts = lambda tmpdir: tmpdir

    try:
        res = bass_utils.run_bass_kernel_spmd(
            nc, in_maps, list(range(NCORES)), trace=trace
        )
    except Exception:
        if not trace:
            raise
        res = bass_utils.run_bass_kernel_spmd(
            nc, in_maps, list(range(NCORES)), trace=False
        )
    _built["last_result"] = res

    out = np.empty((T, B, N), np.float32)
    for c in range(NCORES):
        u8 = np.asarray(res.results[c]["u8"])          # fp8e4 [P, T*F]
        bits = u8.view(np.uint8).reshape(P, T, F).transpose(1, 0, 2)
        spikes = (bits < 0x80).astype(np.float32).reshape(T, BL, N)
        out[:, c * BL:(c + 1) * BL, :] = spikes
    return out


# revision 18
# speedup vs baseline: 1.0639x; 1.0078x over previous
"""LIF (leaky integrate-and-fire) spiking-neuron scan on 8 Trainium2 NeuronCores.

Reference semantics (per element, f32):
    h_t = v_{t-1} + (x_t - v_{t-1}) / 2        (tau = 2, v_reset = 0)
    s_t = (h_t >= 1)                           (spike, threshold v_th = 1)
    v_t = h_t * (1 - s_t)                      (hard reset)

Device formulation: shifted pre-activation u_t = v_{t-1} + x_t - 2, so
s_t = (u_t >= 0) and the whole step is ONE fused custom-DVE op:
    u_t = (0.5 * u_{t-1} + 1) * (u_{t-1} < 0) + (x_t - 2)
(x - 2 is precomputed on host; u_0 = -2 encodes v_0 = 0.)  The custom op
(registered into concourse.dve_ops at import, compiled into the per-NEFF
DVE table) runs at 1 elem/cycle/lane fp32 — one ~691 ns instruction per
timestep, ~727 ns dependent-chain pitch, bit-exact f32 arithmetic.

Output: u_t cast fp32 -> fp8e4 by the (otherwise idle) ACT engine, then
stored via HWDGE.  Spike <=> u_t >= 0 <=> fp8 sign bit clear (sign
survives rounding and saturation), so the host decodes
spikes = (u8_bits < 0x80).  Casting on ACT instead of during the DMA
keeps the 16.8 MB of fp32 u-reads off the SDMA/SBUF-AXI budget: DMA
moves only 16.8 MB in + 4.2 MB out per core vs 46.5 us of DVE time.
Loads get a dedicated HWDGE ring (SP); casts + stores ride the ACT ring.

Sharding: batch dim B=64 split across 8 cores (8 rows each); time stays
local.  DRAM layout is partition-major [128, T*512] so every DMA segment
is contiguous per partition.
"""

import os
import numpy as np

T, B, N = 64, 64, 8192
NCORES = 8
BL = B // NCORES          # batch rows per core
P = 128                   # SBUF partitions
F = (BL * N) // P         # free elems per partition per step  (512)

# timestep chunking: small first chunks prime the pipeline, then steady-state
LOAD_CHUNKS = [1, 1, 2, 4] + [4] * 14
assert sum(LOAD_CHUNKS) == T
UC = 8                    # u-history chunk (timesteps per SBUF u buffer)
CAST = 4                  # timesteps per ACT fp32->fp8 cast op
TAPER = 1                 # cast granularity for the last TAIL steps
TAIL = 4                  # final steps cast one-by-one to shrink the tail

_built = {}


def _register_lif_op():
    from concourse import dve_ops
    from concourse.dve_spec import (
        Spec, Src0, Src1, C0, Zero, One, lower, _has_src1,
    )
    from concourse.dve_uop import DveOpSpec

    for op in dve_ops.OPS:
        if op.name == "LIF_STEP_ANT":
            return op

    body = (Src0 * C0 + One) * (Src0 < Zero) + Src1

    def ref(in0, in1, s0, s1, imm2):
        f = np.float32
        mask = (in0 < 0).astype(f)
        return ((in0 * f(s0) + f(1.0)) * mask + in1).astype(f)

    spec = Spec(body=body, reference=ref)
    name = "LIF_STEP_ANT"
    row = dve_ops._CUSTOM_DVE_ROW_BASE + len(dve_ops.OPS)
    shas = {}
    for ver in ("v3", "v4"):
        tmp = DveOpSpec(
            name=name, opcode=row, uops=lower(spec, ver=ver),
            rd1_en=_has_src1(spec),
        )
        shas[ver] = tmp.sha(ver)
    op = dve_ops.DveOp(name, spec, subdim=False, uops_sha=shas)
    dve_ops.OPS.append(op)
    dve_ops._SUB_OPCODE_FOR_NAME[name] = row
    dve_ops.CUSTOM_DVE_SPECS[name] = spec
    return op


def _build():
    if "nc" in _built:
        return _built["nc"]

    from contextlib import ExitStack
    import concourse.mybir as mybir
    from concourse import bacc, tile

    # Slim the kernel-exit choreography: the stock exit is
    # drain -> all_engine_barrier -> clear sems -> all_engine_barrier; the
    # trailing barrier only orders the sem clears against later instructions,
    # of which there are none at kernel end (~3us saved).
    from concourse.vector_clock import ScopedClock

    def _slim_drain_and_barrier(self, tick_clock, wait_clock):
        drain_inst = self.nc.sync.drain()
        wait_clock.add_sem_waits(
            drain_inst.ins, ScopedClock({None: tick_clock.global_clock})
        )
        self.nc.all_engine_barrier()
        popped = self.nc._tile_sem_poison_stack.pop()
        assert popped is self._sem_poison
        self.nc.clear_and_free_semaphores(list(self.sems.allocated().values()))

    tile.TileContext._drain_and_barrier = _slim_drain_and_barrier

    lif_op = _register_lif_op()

    nc = bacc.Bacc("TRN2", target_bir_lowering=False, debug=False)
    # partition-major layouts: [P, T*F] so per-partition bytes are contiguous
    x_ext = nc.dram_tensor("x", [P, T * F], mybir.dt.float32, kind="ExternalInput")
    u8_ext = nc.dram_tensor("u8", [P, T * F], mybir.dt.float8e4, kind="ExternalOutput")

    Copy = mybir.ActivationFunctionType.Copy

    # scratch target for the store-gate dummy DMA
    gate_ext = nc.dram_tensor("gate", [1, 1], mybir.dt.float32, kind="Internal")

    with tile.TileContext(nc) as tc:
        with ExitStack() as ctx:
            # x chunks and fp8 output tiles are fully resident (unique tags,
            # no recycling): load issues are never gated on buffer reuse, so
            # both HWDGE rings stream the whole 16 MiB input uninterrupted.
            # x chunks recycle through 12 buffers: deep enough that a load
            # issue is never gated on far-future consumption, shallow enough
            # that the issue pacing keeps each HWDGE ring's in-flight queue
            # (~4 transfers deep) from blocking its engine for long.
            xp = ctx.enter_context(tc.tile_pool(name="xp", bufs=12))
            up = ctx.enter_context(tc.tile_pool(name="up", bufs=3))
            cp = ctx.enter_context(tc.tile_pool(name="cp", bufs=1))
            ip = ctx.enter_context(tc.tile_pool(name="ip", bufs=1))

            u0 = ip.tile([P, F], mybir.dt.float32)
            nc.vector.memset(u0[:], -2.0)

            # issue all loads up front, alternating the two HWDGE rings
            x_tiles = []
            t0 = 0
            for i, ch in enumerate(LOAD_CHUNKS):
                xt = xp.tile([P, ch * F], mybir.dt.float32, tag="xchunk")
                dma_eng = nc.sync if i % 2 == 0 else nc.scalar
                dma_eng.dma_start(out=xt[:], in_=x_ext[:, t0 * F:(t0 + ch) * F])
                x_tiles.append((t0, ch, xt))
                t0 += ch

            C8S = 8         # timesteps per fp8 output tile / store
            prev = u0[:]
            uc = None
            c8_tiles = []   # (t_first, tile) pending output stores
            c8 = None
            for (t0, ch, xt) in x_tiles:
                for k in range(ch):
                    t = t0 + k
                    # final steps cast one-by-one so the post-compute tail
                    # is a single small cast + store
                    cast = CAST if t < T - TAIL else TAPER
                    if t % UC == 0:
                        uc = up.tile([P, UC * F], mybir.dt.float32, tag="uchunk")
                    cur = uc[:, (t % UC) * F:(t % UC + 1) * F]
                    nc.vector._custom_dve(
                        lif_op, out=cur, in0=prev,
                        in1=xt[:, k * F:(k + 1) * F], s0=0.5,
                    )
                    prev = cur
                    if t % C8S == 0:
                        c8 = cp.tile([P, C8S * F], mybir.dt.float8e4,
                                     tag=f"c8_{t // C8S}")
                        c8_tiles.append((t, c8))
                    if (t + 1) % cast == 0:
                        # ACT cast fp32 -> fp8e4 (sign-exact; |u| << 448)
                        uq = t % UC
                        cq = t % C8S
                        nc.scalar.activation(
                            c8[:, (cq - cast + 1) * F:(cq + 1) * F],
                            uc[:, (uq - cast + 1) * F:(uq + 1) * F],
                            Copy, bias=0.0, scale=1.0,
                        )

            # store gate: a dummy DMA whose semaphore wait (on the final x
            # chunk's load) blocks the sync queue, deferring every output
            # store until ALL input loads have finished.  Keeps the fp8
            # stores from stealing HBM/SBUF-AXI bandwidth during the load
            # phase; they burst at full rate under the last compute steps.
            last_xt = x_tiles[-1][2]
            nc.sync.dma_start(out=gate_ext[:, :], in_=last_xt[:1, :1])
            for (ct0, ct) in c8_tiles:
                if ct0 + C8S == T:
                    # split the final store so the exit drain only waits on
                    # a small last transfer
                    h = C8S // 2
                    nc.sync.dma_start(
                        out=u8_ext[:, ct0 * F:(ct0 + h) * F], in_=ct[:, :h * F]
                    )
                    nc.sync.dma_start(
                        out=u8_ext[:, (ct0 + h) * F:(ct0 + C8S) * F],
                        in_=ct[:, h * F:],
                    )
                else:
                    nc.sync.dma_start(
                        out=u8_ext[:, ct0 * F:(ct0 + C8S) * F], in_=ct[:],
                    )

    nc.compile()
    _built["nc"] = nc
    return nc


def _install_ntff_hook() -> bool:
    """Provide antenv.axon_hooks (absent in this image) so that
    run_bass_kernel_spmd(trace=True) can capture NTFF profiles via the
    ctypes hook that trn_agent_boot already implements."""
    try:
        from antenv.axon_hooks import get_axon_ntff_profile_hook  # noqa: F401
        return True
    except ImportError:
        pass
    try:
        import sys
        import types
        import antenv
        from trn_agent_boot.trn_boot import _ntff_profile_via_ctypes

        hook = _ntff_profile_via_ctypes("/opt/axon/libaxon_pjrt.so")
        if hook is None:
            return False
        mod = types.ModuleType("antenv.axon_hooks")
        state = {"hook": hook}
        mod.get_axon_ntff_profile_hook = lambda: state["hook"]
        mod.set_axon_ntff_profile_hook = lambda h: state.__setitem__("hook", h)
        sys.modules["antenv.axon_hooks"] = mod
        antenv.axon_hooks = mod
        return True
    except Exception:
        return False


def kernel(x: np.ndarray) -> np.ndarray:
    import concourse.bass_utils as bass_utils

    nc = _build()

    x = np.asarray(x)
    assert x.shape == (T, B, N) and x.dtype == np.float32

    xs = x - np.float32(2.0)
    in_maps = []
    for c in range(NCORES):
        # [T, BL*N] -> [T, P, F] -> [P, T, F] -> [P, T*F]  (partition-major)
        shard = (
            xs[:, c * BL:(c + 1) * BL, :]
            .reshape(T, P, F)
            .transpose(1, 0, 2)
            .reshape(P, T * F)
        )
        in_maps.append({"x": np.ascontiguousarray(shard)})

    trace = bool(int(os.environ.get("LIF_TRACE", "0")))
    if trace:
        trace = _install_ntff_hook()
        # artifact upload has no bucket in this container; neuter it
        bass_utils.upload_artifacts = lambda tmpdir: tmpdir

    try:
        res = bass_utils.run_bass_kernel_spmd(
            nc, in_maps, list(range(NCORES)), trace=trace
        )
    except Exception:
        if not trace:
            raise
        res = bass_utils.run_bass_kernel_spmd(
            nc, in_maps, list(range(NCORES)), trace=False
        )
    _built["last_result"] = res

    out = np.empty((T, B, N), np.float32)
    for c in range(NCORES):
        u8 = np.asarray(res.results[c]["u8"])          # fp8e4 [P, T*F]
        bits = u8.view(np.uint8).reshape(P, T, F).transpose(1, 0, 2)
        spikes = (bits < 0x80).astype(np.float32).reshape(T, BL, N)
        out[:, c * BL:(c + 1) * BL, :] = spikes
    return out


# revision 19
# speedup vs baseline: 1.0933x; 1.0276x over previous
"""LIF (leaky integrate-and-fire) spiking-neuron scan on 8 Trainium2 NeuronCores.

Reference semantics (per element, f32):
    h_t = v_{t-1} + (x_t - v_{t-1}) / 2        (tau = 2, v_reset = 0)
    s_t = (h_t >= 1)                           (spike, threshold v_th = 1)
    v_t = h_t * (1 - s_t)                      (hard reset)

Device formulation: shifted pre-activation u_t = v_{t-1} + x_t - 2, so
s_t = (u_t >= 0) and the whole step is ONE fused custom-DVE op:
    u_t = (0.5 * u_{t-1} + 1) * (u_{t-1} < 0) + (x_t - 2)
(x - 2 is precomputed on host; u_0 = -2 encodes v_0 = 0.)  The custom op
(registered into concourse.dve_ops at import, compiled into the per-NEFF
DVE table) runs at 1 elem/cycle/lane fp32 — one ~691 ns instruction per
timestep, ~727 ns dependent-chain pitch, bit-exact f32 arithmetic.

Output: u_t cast fp32 -> fp8e4 by the (otherwise idle) ACT engine, then
stored via HWDGE.  Spike <=> u_t >= 0 <=> fp8 sign bit clear (sign
survives rounding and saturation), so the host decodes
spikes = (u8_bits < 0x80).  Casting on ACT instead of during the DMA
keeps the 16.8 MB of fp32 u-reads off the SDMA/SBUF-AXI budget: DMA
moves only 16.8 MB in + 4.2 MB out per core vs 46.5 us of DVE time.
Loads get a dedicated HWDGE ring (SP); casts + stores ride the ACT ring.

Sharding: batch dim B=64 split across 8 cores (8 rows each); time stays
local.  DRAM layout is partition-major [128, T*512] so every DMA segment
is contiguous per partition.
"""

import os
import numpy as np

T, B, N = 64, 64, 8192
NCORES = 8
BL = B // NCORES          # batch rows per core
P = 128                   # SBUF partitions
F = (BL * N) // P         # free elems per partition per step  (512)

# timestep chunking: small first chunks prime the pipeline, then steady-state
LOAD_CHUNKS = [1, 1, 2, 4] + [4] * 14
assert sum(LOAD_CHUNKS) == T
UC = 8                    # u-history chunk (timesteps per SBUF u buffer)
CAST = 4                  # timesteps per ACT fp32->fp8 cast op
TAPER = 1                 # cast granularity for the last TAIL steps
TAIL = 4                  # final steps cast one-by-one to shrink the tail

_built = {}


def _register_lif_op():
    from concourse import dve_ops
    from concourse.dve_spec import (
        Spec, Src0, Src1, C0, Zero, One, lower, _has_src1,
    )
    from concourse.dve_uop import DveOpSpec

    for op in dve_ops.OPS:
        if op.name == "LIF_STEP_ANT":
            return op

    body = (Src0 * C0 + One) * (Src0 < Zero) + Src1

    def ref(in0, in1, s0, s1, imm2):
        f = np.float32
        mask = (in0 < 0).astype(f)
        return ((in0 * f(s0) + f(1.0)) * mask + in1).astype(f)

    spec = Spec(body=body, reference=ref)
    name = "LIF_STEP_ANT"
    row = dve_ops._CUSTOM_DVE_ROW_BASE + len(dve_ops.OPS)
    shas = {}
    for ver in ("v3", "v4"):
        tmp = DveOpSpec(
            name=name, opcode=row, uops=lower(spec, ver=ver),
            rd1_en=_has_src1(spec),
        )
        shas[ver] = tmp.sha(ver)
    op = dve_ops.DveOp(name, spec, subdim=False, uops_sha=shas)
    dve_ops.OPS.append(op)
    dve_ops._SUB_OPCODE_FOR_NAME[name] = row
    dve_ops.CUSTOM_DVE_SPECS[name] = spec
    return op


def _build():
    if "nc" in _built:
        return _built["nc"]

    from contextlib import ExitStack
    import concourse.mybir as mybir
    from concourse import bacc, tile

    # Slim the kernel-exit choreography: the stock exit is
    # drain -> all_engine_barrier -> clear sems -> all_engine_barrier; the
    # trailing barrier only orders the sem clears against later instructions,
    # of which there are none at kernel end (~3us saved).
    from concourse.vector_clock import ScopedClock

    def _slim_drain_and_barrier(self, tick_clock, wait_clock):
        drain_inst = self.nc.sync.drain()
        wait_clock.add_sem_waits(
            drain_inst.ins, ScopedClock({None: tick_clock.global_clock})
        )
        self.nc.all_engine_barrier()
        popped = self.nc._tile_sem_poison_stack.pop()
        assert popped is self._sem_poison
        self.nc.clear_and_free_semaphores(list(self.sems.allocated().values()))

    tile.TileContext._drain_and_barrier = _slim_drain_and_barrier

    lif_op = _register_lif_op()

    nc = bacc.Bacc("TRN2", target_bir_lowering=False, debug=False)
    # partition-major layouts: [P, T*F] so per-partition bytes are contiguous
    x_ext = nc.dram_tensor("x", [P, T * F], mybir.dt.float32, kind="ExternalInput")
    u8_ext = nc.dram_tensor("u8", [P, T * F], mybir.dt.float8e4, kind="ExternalOutput")

    Copy = mybir.ActivationFunctionType.Copy

    # scratch target for the store-gate dummy DMA
    gate_ext = nc.dram_tensor("gate", [1, 1], mybir.dt.float32, kind="Internal")

    with tile.TileContext(nc) as tc:
        with ExitStack() as ctx:
            # x chunks and fp8 output tiles are fully resident (unique tags,
            # no recycling): load issues are never gated on buffer reuse, so
            # both HWDGE rings stream the whole 16 MiB input uninterrupted.
            # x chunks recycle through 12 buffers: deep enough that a load
            # issue is never gated on far-future consumption, shallow enough
            # that the issue pacing keeps each HWDGE ring's in-flight queue
            # (~4 transfers deep) from blocking its engine for long.
            xp = ctx.enter_context(tc.tile_pool(name="xp", bufs=11))
            up = ctx.enter_context(tc.tile_pool(name="up", bufs=5))
            cp = ctx.enter_context(tc.tile_pool(name="cp", bufs=1))
            ip = ctx.enter_context(tc.tile_pool(name="ip", bufs=1))

            u0 = ip.tile([P, F], mybir.dt.float32)
            nc.vector.memset(u0[:], -2.0)

            # issue all loads up front, alternating the two HWDGE rings
            x_tiles = []
            t0 = 0
            for i, ch in enumerate(LOAD_CHUNKS):
                xt = xp.tile([P, ch * F], mybir.dt.float32, tag="xchunk")
                dma_eng = nc.sync if i % 2 == 0 else nc.scalar
                dma_eng.dma_start(out=xt[:], in_=x_ext[:, t0 * F:(t0 + ch) * F])
                x_tiles.append((t0, ch, xt))
                t0 += ch

            C8S = 8         # timesteps per fp8 output tile / store
            prev = u0[:]
            uc = None
            c8_tiles = []   # (t_first, tile) pending output stores
            c8 = None
            for (t0, ch, xt) in x_tiles:
                for k in range(ch):
                    t = t0 + k
                    # final steps cast one-by-one so the post-compute tail
                    # is a single small cast + store
                    cast = CAST if t < T - TAIL else TAPER
                    if t % UC == 0:
                        uc = up.tile([P, UC * F], mybir.dt.float32, tag="uchunk")
                    cur = uc[:, (t % UC) * F:(t % UC + 1) * F]
                    nc.vector._custom_dve(
                        lif_op, out=cur, in0=prev,
                        in1=xt[:, k * F:(k + 1) * F], s0=0.5,
                    )
                    prev = cur
                    if t % C8S == 0:
                        c8 = cp.tile([P, C8S * F], mybir.dt.float8e4,
                                     tag=f"c8_{t // C8S}")
                        c8_tiles.append((t, c8))
                    if (t + 1) % cast == 0:
                        # ACT cast fp32 -> fp8e4 (sign-exact; |u| << 448)
                        uq = t % UC
                        cq = t % C8S
                        nc.scalar.activation(
                            c8[:, (cq - cast + 1) * F:(cq + 1) * F],
                            uc[:, (uq - cast + 1) * F:(uq + 1) * F],
                            Copy, bias=0.0, scale=1.0,
                        )

            # store gate: a dummy DMA whose semaphore wait (on the final x
            # chunk's load) blocks the sync queue, deferring every output
            # store until ALL input loads have finished.  Keeps the fp8
            # stores from stealing HBM/SBUF-AXI bandwidth during the load
            # phase; they burst at full rate under the last compute steps.
            last_xt = x_tiles[-1][2]
            nc.sync.dma_start(out=gate_ext[:, :], in_=last_xt[:1, :1])
            for (ct0, ct) in c8_tiles:
                if ct0 + C8S == T:
                    # split the final store so the exit drain only waits on
                    # a small last transfer
                    h = C8S // 2
                    nc.sync.dma_start(
                        out=u8_ext[:, ct0 * F:(ct0 + h) * F], in_=ct[:, :h * F]
                    )
                    nc.sync.dma_start(
                        out=u8_ext[:, (ct0 + h) * F:(ct0 + C8S) * F],
                        in_=ct[:, h * F:],
                    )
                else:
                    nc.sync.dma_start(
                        out=u8_ext[:, ct0 * F:(ct0 + C8S) * F], in_=ct[:],
                    )

    nc.compile()
    _built["nc"] = nc
    return nc


def _install_ntff_hook() -> bool:
    """Provide antenv.axon_hooks (absent in this image) so that
    run_bass_kernel_spmd(trace=True) can capture NTFF profiles via the
    ctypes hook that trn_agent_boot already implements."""
    try:
        from antenv.axon_hooks import get_axon_ntff_profile_hook  # noqa: F401
        return True
    except ImportError:
        pass
    try:
        import sys
        import types
        import antenv
        from trn_agent_boot.trn_boot import _ntff_profile_via_ctypes

        hook = _ntff_profile_via_ctypes("/opt/axon/libaxon_pjrt.so")
        if hook is None:
            return False
        mod = types.ModuleType("antenv.axon_hooks")
        state = {"hook": hook}
        mod.get_axon_ntff_profile_hook = lambda: state["hook"]
        mod.set_axon_ntff_profile_hook = lambda h: state.__setitem__("hook", h)
        sys.modules["antenv.axon_hooks"] = mod
        antenv.axon_hooks = mod
        return True
    except Exception:
        return False


def kernel(x: np.ndarray) -> np.ndarray:
    import concourse.bass_utils as bass_utils

    nc = _build()

    x = np.asarray(x)
    assert x.shape == (T, B, N) and x.dtype == np.float32

    xs = x - np.float32(2.0)
    in_maps = []
    for c in range(NCORES):
        # [T, BL*N] -> [T, P, F] -> [P, T, F] -> [P, T*F]  (partition-major)
        shard = (
            xs[:, c * BL:(c + 1) * BL, :]
            .reshape(T, P, F)
            .transpose(1, 0, 2)
            .reshape(P, T * F)
        )
        in_maps.append({"x": np.ascontiguousarray(shard)})

    trace = bool(int(os.environ.get("LIF_TRACE", "0")))
    if trace:
        trace = _install_ntff_hook()
        # artifact upload has no bucket in this container; neuter it
        bass_utils.upload_artifacts = lambda tmpdir: tmpdir

    try:
        res = bass_utils.run_bass_kernel_spmd(
            nc, in_maps, list(range(NCORES)), trace=trace
        )
    except Exception:
        if not trace:
            raise
        res = bass_utils.run_bass_kernel_spmd(
            nc, in_maps, list(range(NCORES)), trace=False
        )
    _built["last_result"] = res

    out = np.empty((T, B, N), np.float32)
    for c in range(NCORES):
        u8 = np.asarray(res.results[c]["u8"])          # fp8e4 [P, T*F]
        bits = u8.view(np.uint8).reshape(P, T, F).transpose(1, 0, 2)
        spikes = (bits < 0x80).astype(np.float32).reshape(T, BL, N)
        out[:, c * BL:(c + 1) * BL, :] = spikes
    return out


# revision 21
# speedup vs baseline: 1.1728x; 1.0727x over previous
"""LIF (leaky integrate-and-fire) spiking-neuron scan on 8 Trainium2 NeuronCores.

Reference semantics (per element, f32):
    h_t = v_{t-1} + (x_t - v_{t-1}) / 2        (tau = 2, v_reset = 0)
    s_t = (h_t >= 1)                           (spike, threshold v_th = 1)
    v_t = h_t * (1 - s_t)                      (hard reset)

Device formulation: shifted pre-activation u_t = v_{t-1} + x_t - 2, so
s_t = (u_t >= 0) and the whole step is ONE fused custom-DVE op:
    u_t = (0.5 * u_{t-1} + 1) * (u_{t-1} < 0) + (x_t - 2)
(x - 2 is precomputed on host; u_0 = -2 encodes v_0 = 0.)  The custom op
(registered into concourse.dve_ops at import, compiled into the per-NEFF
DVE table) runs at 1 elem/cycle/lane fp32 — one ~691 ns instruction per
timestep, ~727 ns dependent-chain pitch, bit-exact f32 arithmetic.

Output: u_t cast fp32 -> fp8e4 by the (otherwise idle) ACT engine, then
stored via HWDGE.  Spike <=> u_t >= 0 <=> fp8 sign bit clear (sign
survives rounding and saturation), so the host decodes
spikes = (u8_bits < 0x80).  Casting on ACT instead of during the DMA
keeps the 16.8 MB of fp32 u-reads off the SDMA/SBUF-AXI budget: DMA
moves only 16.8 MB in + 4.2 MB out per core vs 46.5 us of DVE time.
Loads get a dedicated HWDGE ring (SP); casts + stores ride the ACT ring.

Sharding: batch dim B=64 split across 8 cores (8 rows each); time stays
local.  DRAM layout is partition-major [128, T*512] so every DMA segment
is contiguous per partition.
"""

import os
import numpy as np

T, B, N = 64, 64, 8192
NCORES = 8
BL = B // NCORES          # batch rows per core
P = 128                   # SBUF partitions
F = (BL * N) // P         # free elems per partition per step  (512)

# timestep chunking: small first chunks prime the pipeline, then steady-state
# (even count and symmetric sizes keep the two HWDGE rings byte-balanced)
LOAD_CHUNKS = [1, 1, 1, 1, 2, 2] + [4] * 14
assert sum(LOAD_CHUNKS) == T
UC = 8                    # u-history chunk (timesteps per SBUF u buffer)
CAST = 4                  # timesteps per ACT fp32->fp8 cast op
TAPER = 1                 # cast granularity for the last TAIL steps
TAIL = 4                  # final steps cast one-by-one to shrink the tail

_built = {}


def _register_lif_op():
    from concourse import dve_ops
    from concourse.dve_spec import (
        Spec, Src0, Src1, C0, Zero, One, lower, _has_src1,
    )
    from concourse.dve_uop import DveOpSpec

    for op in dve_ops.OPS:
        if op.name == "LIF_STEP_ANT":
            return op

    body = (Src0 * C0 + One) * (Src0 < Zero) + Src1

    def ref(in0, in1, s0, s1, imm2):
        f = np.float32
        mask = (in0 < 0).astype(f)
        return ((in0 * f(s0) + f(1.0)) * mask + in1).astype(f)

    spec = Spec(body=body, reference=ref)
    name = "LIF_STEP_ANT"
    row = dve_ops._CUSTOM_DVE_ROW_BASE + len(dve_ops.OPS)
    shas = {}
    for ver in ("v3", "v4"):
        tmp = DveOpSpec(
            name=name, opcode=row, uops=lower(spec, ver=ver),
            rd1_en=_has_src1(spec),
        )
        shas[ver] = tmp.sha(ver)
    op = dve_ops.DveOp(name, spec, subdim=False, uops_sha=shas)
    dve_ops.OPS.append(op)
    dve_ops._SUB_OPCODE_FOR_NAME[name] = row
    dve_ops.CUSTOM_DVE_SPECS[name] = spec
    return op


def _build():
    if "nc" in _built:
        return _built["nc"]

    from contextlib import ExitStack
    import concourse.mybir as mybir
    from concourse import bacc, tile

    # Slim the kernel-exit choreography: the stock exit is
    # drain -> all_engine_barrier -> clear sems -> all_engine_barrier; the
    # trailing barrier only orders the sem clears against later instructions,
    # of which there are none at kernel end (~3us saved).
    from concourse.vector_clock import ScopedClock

    def _slim_drain_and_barrier(self, tick_clock, wait_clock):
        drain_inst = self.nc.sync.drain()
        wait_clock.add_sem_waits(
            drain_inst.ins, ScopedClock({None: tick_clock.global_clock})
        )
        self.nc.all_engine_barrier()
        popped = self.nc._tile_sem_poison_stack.pop()
        assert popped is self._sem_poison
        self.nc.clear_and_free_semaphores(list(self.sems.allocated().values()))

    tile.TileContext._drain_and_barrier = _slim_drain_and_barrier

    lif_op = _register_lif_op()

    nc = bacc.Bacc("TRN2", target_bir_lowering=False, debug=False)
    # partition-major layouts: [P, T*F] so per-partition bytes are contiguous
    x_ext = nc.dram_tensor("x", [P, T * F], mybir.dt.float32, kind="ExternalInput")
    u8_ext = nc.dram_tensor("u8", [P, T * F], mybir.dt.float8e4, kind="ExternalOutput")

    Copy = mybir.ActivationFunctionType.Copy

    # scratch target for the store-gate dummy DMA
    gate_ext = nc.dram_tensor("gate", [1, 1], mybir.dt.float32, kind="Internal")

    with tile.TileContext(nc) as tc:
        with ExitStack() as ctx:
            # x chunks and fp8 output tiles are fully resident (unique tags,
            # no recycling): load issues are never gated on buffer reuse, so
            # both HWDGE rings stream the whole 16 MiB input uninterrupted.
            # x chunks recycle through 12 buffers: deep enough that a load
            # issue is never gated on far-future consumption, shallow enough
            # that the issue pacing keeps each HWDGE ring's in-flight queue
            # (~4 transfers deep) from blocking its engine for long.
            xp = ctx.enter_context(tc.tile_pool(name="xp", bufs=11))
            up = ctx.enter_context(tc.tile_pool(name="up", bufs=5))
            cp = ctx.enter_context(tc.tile_pool(name="cp", bufs=1))
            ip = ctx.enter_context(tc.tile_pool(name="ip", bufs=1))

            u0 = ip.tile([P, F], mybir.dt.float32)
            nc.vector.memset(u0[:], -2.0)

            # issue all loads up front, alternating the two HWDGE rings
            x_tiles = []
            t0 = 0
            for i, ch in enumerate(LOAD_CHUNKS):
                xt = xp.tile([P, ch * F], mybir.dt.float32, tag="xchunk")
                dma_eng = nc.sync if i % 2 == 0 else nc.scalar
                dma_eng.dma_start(out=xt[:], in_=x_ext[:, t0 * F:(t0 + ch) * F])
                x_tiles.append((t0, ch, xt))
                t0 += ch

            C8S = 8         # timesteps per fp8 output tile / store
            prev = u0[:]
            uc = None
            c8_tiles = []   # (t_first, tile) pending output stores
            c8 = None
            for (t0, ch, xt) in x_tiles:
                for k in range(ch):
                    t = t0 + k
                    # final steps cast one-by-one so the post-compute tail
                    # is a single small cast + store
                    cast = CAST if t < T - TAIL else TAPER
                    if t % UC == 0:
                        uc = up.tile([P, UC * F], mybir.dt.float32, tag="uchunk")
                    cur = uc[:, (t % UC) * F:(t % UC + 1) * F]
                    nc.vector._custom_dve(
                        lif_op, out=cur, in0=prev,
                        in1=xt[:, k * F:(k + 1) * F], s0=0.5,
                    )
                    prev = cur
                    if t % C8S == 0:
                        c8 = cp.tile([P, C8S * F], mybir.dt.float8e4,
                                     tag=f"c8_{t // C8S}")
                        c8_tiles.append((t, c8))
                    if (t + 1) % cast == 0:
                        # ACT cast fp32 -> fp8e4 (sign-exact; |u| << 448)
                        uq = t % UC
                        cq = t % C8S
                        nc.scalar.activation(
                            c8[:, (cq - cast + 1) * F:(cq + 1) * F],
                            uc[:, (uq - cast + 1) * F:(uq + 1) * F],
                            Copy, bias=0.0, scale=1.0,
                        )

            # store gate: a dummy SWDGE DMA whose semaphore wait (on a late
            # x chunk's load) blocks the gpsimd queue, deferring the output
            # stores until the input load phase is nearly done.  The stores
            # live on the otherwise-idle gpsimd (SWDGE) queue: they issue
            # independently of the ACT cast stream and burst on their own
            # SDMA queue row without competing with mid-kernel loads.
            gate_xt = x_tiles[-4][2]
            nc.gpsimd.dma_start(out=gate_ext[:, :], in_=gate_xt[:1, :1])
            for (ct0, ct) in c8_tiles:
                if ct0 + C8S == T:
                    # split the final store so the exit drain only waits on
                    # a tiny last transfer
                    h = C8S - 1
                    nc.gpsimd.dma_start(
                        out=u8_ext[:, ct0 * F:(ct0 + h) * F], in_=ct[:, :h * F]
                    )
                    nc.gpsimd.dma_start(
                        out=u8_ext[:, (ct0 + h) * F:(ct0 + C8S) * F],
                        in_=ct[:, h * F:],
                    )
                else:
                    nc.gpsimd.dma_start(
                        out=u8_ext[:, ct0 * F:(ct0 + C8S) * F], in_=ct[:],
                    )

    nc.compile()
    _built["nc"] = nc
    return nc


def _install_ntff_hook() -> bool:
    """Provide antenv.axon_hooks (absent in this image) so that
    run_bass_kernel_spmd(trace=True) can capture NTFF profiles via the
    ctypes hook that trn_agent_boot already implements."""
    try:
        from antenv.axon_hooks import get_axon_ntff_profile_hook  # noqa: F401
        return True
    except ImportError:
        pass
    try:
        import sys
        import types
        import antenv
        from trn_agent_boot.trn_boot import _ntff_profile_via_ctypes

        hook = _ntff_profile_via_ctypes("/opt/axon/libaxon_pjrt.so")
        if hook is None:
            return False
        mod = types.ModuleType("antenv.axon_hooks")
        state = {"hook": hook}
        mod.get_axon_ntff_profile_hook = lambda: state["hook"]
        mod.set_axon_ntff_profile_hook = lambda h: state.__setitem__("hook", h)
        sys.modules["antenv.axon_hooks"] = mod
        antenv.axon_hooks = mod
        return True
    except Exception:
        return False


def kernel(x: np.ndarray) -> np.ndarray:
    import concourse.bass_utils as bass_utils

    nc = _build()

    x = np.asarray(x)
    assert x.shape == (T, B, N) and x.dtype == np.float32

    xs = x - np.float32(2.0)
    in_maps = []
    for c in range(NCORES):
        # [T, BL*N] -> [T, P, F] -> [P, T, F] -> [P, T*F]  (partition-major)
        shard = (
            xs[:, c * BL:(c + 1) * BL, :]
            .reshape(T, P, F)
            .transpose(1, 0, 2)
            .reshape(P, T * F)
        )
        in_maps.append({"x": np.ascontiguousarray(shard)})

    trace = bool(int(os.environ.get("LIF_TRACE", "0")))
    if trace:
        trace = _install_ntff_hook()
        # artifact upload has no bucket in this container; neuter it
        bass_utils.upload_artifacts = lambda tmpdir: tmpdir

    try:
        res = bass_utils.run_bass_kernel_spmd(
            nc, in_maps, list(range(NCORES)), trace=trace
        )
    except Exception:
        if not trace:
            raise
        res = bass_utils.run_bass_kernel_spmd(
            nc, in_maps, list(range(NCORES)), trace=False
        )
    _built["last_result"] = res

    out = np.empty((T, B, N), np.float32)
    for c in range(NCORES):
        u8 = np.asarray(res.results[c]["u8"])          # fp8e4 [P, T*F]
        bits = u8.view(np.uint8).reshape(P, T, F).transpose(1, 0, 2)
        spikes = (bits < 0x80).astype(np.float32).reshape(T, BL, N)
        out[:, c * BL:(c + 1) * BL, :] = spikes
    return out
